# revision 18
# baseline (speedup 1.0000x reference)
"""Trainium2 Bass kernel for nn_EquivariantMultiheadAttention.

Sharding: query-point axis (dim 1) split across 8 cores (16 points each).

Structural optimizations vs the straightforward mapping:

1. ky branch as a rank-R separable expansion.  The ky-MLP is a smooth
   function of two scalars (f_key, f_query) per (batch, channel); host
   fits silu(MLP_y(fk,fq)) ~= sum_r u_r(fk) v_r(fq) via SVD on a 1-D
   grid (cubic-spline eval at data points).  On device the whole ky
   branch is ONE fp32 matmul (K = C*R+1) per 32-query-element group.
   The extra rank row carries -30*(1-mask_k), folding the key mask into
   the logits so exp() of masked keys ~ 0.

2. kg branch exact, PE-tiling aware:
   - L1 (K=9): two row-tiled matmuls per 2-tile chunk (tile_position
     (0,0)/(32,0), banded rhs) -> ~2x stream concurrency.
   - L2 (K=128 block-diag): dense matmuls, N=512 each.
   - L3 (M=32): 4-way col-tiled quads (tile_position (0,32cg),
     cg = u%4) emitted per chunk-pair -> ~4x stream concurrency.
   - Activations as [128, 1024] instructions to amortize ACT overhead.

3. Phase 2 (Exp table): exp with accum_out gives den = sum(e) free;
   num = reduce(e * fkeym) on the vector engine; residual + query mask;
   [128, 4] result.  w_out applied host-side.
"""
import numpy as np
import ml_dtypes

BF16 = ml_dtypes.bfloat16

B, N, S, DG, C, HID, COUT = 2, 128, 4, 8, 4, 32, 8
NCORE = 8
QL = N // NCORE          # 16 query points per core
KEY = N * S              # 512 keys
T = B * QL * S           # 128 tiles (query elements) per core
RK = 12                  # ky separable rank
KRANK = C * RK + 1       # 49 (last row = mask fold)
GRID = 161               # fit grid points
NCH = T // 2             # 64 two-tile chunks

_PROG = None


def _silu_np(v):
    return v / (1.0 + np.exp(-v))


def _mlp_np(x, W1, b1, W2, b2, W3, b3):
    h = _silu_np(x @ W1.T + b1)
    h = _silu_np(h @ W2.T + b2)
    return _silu_np(h @ W3.T + b3)


def _spline_eval(xg, yg, x):
    """Natural cubic spline through uniform grid (xg, yg), evaluated at x."""
    n = len(xg)
    h = float(xg[1] - xg[0])
    d = 6.0 / (h * h) * (yg[:-2] - 2.0 * yg[1:-1] + yg[2:])
    m = np.zeros(n, np.float64)
    cp = np.zeros(n - 2, np.float64)
    dp = np.zeros(n - 2, np.float64)
    cp[0] = 0.25
    dp[0] = d[0] * 0.25
    for i in range(1, n - 2):
        den = 4.0 - cp[i - 1]
        cp[i] = 1.0 / den
        dp[i] = (d[i] - dp[i - 1]) / den
    m[n - 2] = dp[-1]
    for i in range(n - 3, 0, -1):
        m[i] = dp[i - 1] - cp[i - 1] * m[i + 1]
    idx = np.clip(((x - xg[0]) / h).astype(np.int64), 0, n - 2)
    t = x - xg[idx]
    a = yg[idx]
    b_ = (yg[idx + 1] - yg[idx]) / h - h * (2.0 * m[idx] + m[idx + 1]) / 6.0
    c_ = m[idx] / 2.0
    dd = (m[idx + 1] - m[idx]) / (6.0 * h)
    return a + t * (b_ + t * (c_ + t * dd))


def _fit_ky(inp, cf):
    """Rank-RK separable factors of silu(MLP_y) per (batch, channel)."""
    ubank = np.zeros((B, C, RK, KEY), np.float32)
    vq = np.zeros((B, C, RK, N * S), np.float32)
    for b in range(B):
        for c in range(C):
            f = cf[b, :, :, c].reshape(-1).astype(np.float64)
            lo, hi = f.min(), f.max()
            pad = 0.05 * (hi - lo)
            grid = np.linspace(lo - pad, hi + pad, GRID)
            X, Y = np.meshgrid(grid, grid, indexing="ij")
            G = _mlp_np(
                np.stack([X.ravel(), Y.ravel()], -1),
                inp["ky_W1"][c], inp["ky_b1"][c], inp["ky_W2"][c],
                inp["ky_b2"][c], inp["ky_W3"][c], inp["ky_b3"][c],
            ).reshape(GRID, GRID)
            U, sv, Vt = np.linalg.svd(G)
            for r in range(RK):
                ubank[b, c, r] = _spline_eval(grid, U[:, r] * sv[r], f)
                vq[b, c, r] = _spline_eval(grid, Vt[r], f)
    return ubank, vq


def _row_of(u, c):
    """PSUM row of (tile-in-group u, channel c): 4-way col-group interleave."""
    return 32 * (u % 4) + 4 * (u // 4) + c


def _pack_globals(inp):
    cf = np.ascontiguousarray(np.asarray(inp["coset_functions"], np.float32))
    mask = np.asarray(inp["mask"]).astype(np.float32)
    out = {}

    kgW1 = np.asarray(inp["kg_W1"], np.float32)
    w1g = np.zeros((DG + 1, 128), np.float32)
    for c in range(C):
        w1g[0:DG, c * 32:(c + 1) * 32] = kgW1[c].T
    w1g[DG, :] = np.asarray(inp["kg_b1"], np.float32).reshape(128)
    w1gdup = np.zeros((128, 128), np.float32)
    for e in range(4):
        w1gdup[32 * e:32 * e + DG + 1] = w1g
    out["w1gdup"] = w1gdup.astype(BF16)

    W2 = np.asarray(inp["kg_W2"], np.float32)
    L = np.zeros((128, 128), np.float32)
    for c in range(C):
        L[c * 32:(c + 1) * 32, c * 32:(c + 1) * 32] = W2[c].T
    out["w2g"] = L.astype(BF16)

    W3g = np.asarray(inp["kg_W3"], np.float32)
    w3g = np.zeros((128, 256), np.float32)
    for s in range(8):
        for c in range(C):
            w3g[c * 32:(c + 1) * 32, 32 * s + 4 * s + c] = W3g[c, 0, :]
    out["w3g"] = w3g.astype(BF16)

    ubank, vq = _fit_ky(inp, cf)
    bkey = np.zeros((B, KRANK, KEY), np.float32)
    bkey[:, 0:C * RK, :] = ubank.reshape(B, C * RK, KEY)
    mk = mask.reshape(B, KEY)
    bkey[:, C * RK, :] = -30.0 * (1.0 - mk)
    bh = bkey.astype(BF16)
    bl = (bkey - bh.astype(np.float32)).astype(BF16)
    bkey2 = np.concatenate([bh, bl], axis=2)            # [B, KRANK, 2*KEY]
    out["bkey2"] = bkey2

    fkeym = np.zeros((B, 128, KEY), np.float32)
    for row in range(128):
        c = row % 4
        fkeym[:, row, :] = mk * cf[:, :, :, c].reshape(B, KEY)
    out["fkeym"] = fkeym.astype(BF16)
    return out, vq, cf, mask


def _pack_core(core, inp, vq, cf, mask):
    g = np.asarray(inp["pairwise_g"], np.float32)
    qs = slice(core * QL, (core + 1) * QL)
    out = {}
    # g4 [18, NCH*512]: rows 0-8 even tile (g dims + ones), rows 9-17 odd tile
    gt = g[:, qs].transpose(0, 1, 3, 5, 2, 4).reshape(T, DG, KEY)
    g4 = np.empty((18, NCH * KEY), np.float32)
    g4[0:DG] = gt[0::2].transpose(1, 0, 2).reshape(DG, NCH * KEY)
    g4[DG] = 1.0
    g4[9:9 + DG] = gt[1::2].transpose(1, 0, 2).reshape(DG, NCH * KEY)
    g4[9 + DG] = 1.0
    out["g4"] = g4.astype(BF16)

    cfq = cf[:, qs]                                      # [B,QL,S,C]
    maskq = mask[:, qs]                                  # [B,QL,S]
    b2g = np.asarray(inp["kg_b2"], np.float32).reshape(128)
    b3 = np.asarray(inp["kg_b3"], np.float32).reshape(C)

    lhsa = np.zeros((KRANK, 4 * 128), np.float32)
    lhsa[C * RK, :] = 1.0
    small = np.zeros((128, 10), np.float32)
    small[:, 0] = b2g
    for gi in range(4):
        b = gi // 2
        for u in range(32):
            t = 32 * gi + u
            ql, sq = (t % 64) // 4, t % 4
            row = _row_of(u, 0)
            qel = (core * QL + ql) * S + sq
            for c in range(C):
                lhsa[c * RK:(c + 1) * RK, gi * 128 + row + c] = vq[b, c, :, qel]
                small[row + c, 1] = b3[c]
                small[row + c, 2 + gi] = cfq[b, ql, sq, c]
                small[row + c, 6 + gi] = maskq[b, ql, sq]
    ah = lhsa.astype(BF16)
    al = (lhsa - ah.astype(np.float32)).astype(BF16)
    out["lhsa2"] = np.concatenate([ah, al], axis=1)     # [KRANK, 2*512]
    out["small"] = small
    return out


def _build_program():
    from contextlib import ExitStack
    import concourse.bass as bass
    import concourse.tile as tile
    import concourse.mybir as mybir
    from concourse import bacc
    import bass_rust

    f32 = mybir.dt.float32
    bf16 = mybir.dt.bfloat16
    AF = mybir.ActivationFunctionType
    ALU = mybir.AluOpType

    nc = bacc.Bacc("TRN2", target_bir_lowering=False, debug=False,
                   enable_asserts=False, num_devices=NCORE)

    din = {}
    for name, shape, dt in (
        ("g4", [18, NCH * KEY], bf16),
        ("w1gdup", [128, 128], bf16),
        ("w2g", [128, 128], bf16),
        ("w3g", [128, 256], bf16),
        ("bkey2", [B, KRANK, 2 * KEY], bf16),
        ("lhsa2", [KRANK, 2 * 4 * 128], bf16),
        ("small", [128, 10], f32),
        ("fkeym", [B, 128, KEY], bf16),
    ):
        din[name] = nc.dram_tensor(name, shape, dt, kind="ExternalInput").ap()
    dout = nc.dram_tensor("out128", [128, 4], f32, kind="ExternalOutput").ap()

    with tile.TileContext(nc) as tc, ExitStack() as ctx:
        const = ctx.enter_context(tc.tile_pool(name="const", bufs=1))
        gp = ctx.enter_context(tc.tile_pool(name="gp", bufs=4))
        hp = ctx.enter_context(tc.tile_pool(name="hp", bufs=2))
        ps = ctx.enter_context(tc.tile_pool(name="ps", bufs=1, space="PSUM"))
        ep = ctx.enter_context(tc.tile_pool(name="ep", bufs=2))

        # --- constants to SBUF ---
        w1g_s = const.tile([128, 128], bf16, name="w1g_s")
        nc.gpsimd.dma_start(w1g_s[:], din["w1gdup"][:])

        lhsa_s = const.tile([KRANK, 2 * 4 * 128], bf16, name="lhsa_s")
        bkey_s = const.tile([KRANK, B * 2 * KEY], bf16, name="bkey_s")
        fkeym_s = const.tile([128, B * KEY], bf16, name="fkeym_s")
        for b in range(B):
            nc.gpsimd.dma_start(bkey_s[:, b * 2 * KEY:(b + 1) * 2 * KEY],
                                din["bkey2"][b])
        nc.gpsimd.dma_start(lhsa_s[:], din["lhsa2"][:])
        for b in range(B):
            nc.gpsimd.dma_start(fkeym_s[:, b * KEY:(b + 1) * KEY], din["fkeym"][b])
        w2g_s = const.tile([128, 128], bf16, name="w2g_s")
        w3g_s = const.tile([128, 256], bf16, name="w3g_s")
        small_s = const.tile([128, 10], f32, name="small_s")
        ty_s = const.tile([128, 4 * KEY], f32, name="ty_s")
        logits = const.tile([128, 4 * KEY], f32, name="logits")
        out_s = const.tile([128, 4], f32, name="out_s")

        # --- main loop: kg MLP, software-pipelined 2-tile chunks ---
        def rank_all():
            # ky rank matmuls into p2-ring tiles (2 groups per tile)
            for half in range(2):
                Yr = ps.tile([128, 1024], f32, tag="p2", bufs=2, name="Yr")
                for q in range(2):
                    gi = 2 * half + q
                    bb = gi // 2
                    for ai, bi, st, sp in ((0, 0, True, False),
                                           (0, 1, False, False),
                                           (1, 0, False, True)):
                        nc.tensor.matmul(
                            Yr[:, q * KEY:(q + 1) * KEY],
                            lhsa_s[:, ai * 512 + gi * 128:
                                   ai * 512 + (gi + 1) * 128],
                            bkey_s[:, (2 * bb + bi) * KEY:
                                   (2 * bb + bi + 1) * KEY],
                            start=st, stop=sp, tile_position=(0, 0))
                for q in range(2):
                    gi = 2 * half + q
                    nc.vector.tensor_copy(ty_s[:, gi * KEY:(gi + 1) * KEY],
                                          Yr[:, q * KEY:(q + 1) * KEY])

        gts = {}

        def dma_stage(c):
            gt = gp.tile([41, KEY], bf16, tag="gt", bufs=6, name="gt")
            nc.sync.dma_start(gt[0:9, :], din["g4"][0:9, c * KEY:(c + 1) * KEY])
            nc.sync.dma_start(gt[32:41, :],
                              din["g4"][9:18, c * KEY:(c + 1) * KEY])
            gts[c] = gt

        h1live = {}
        h2s = {}
        Xs = {}
        state = {"last": None, "sgacc": None}

        def l1q_stage(k):
            # L1 for chunks 2k, 2k+1 (tiles 4k..4k+3) + fused [128,2048] ACT
            gta = gts.pop(2 * k)
            gtb = gts.pop(2 * k + 1)
            X = ps.tile([128, 2048], f32, tag="px", bufs=1, name="X")
            nc.tensor.matmul(X[:, 0:512], w1g_s[0:9, :], gta[0:9, :],
                             start=True, stop=True, tile_position=(0, 0))
            nc.tensor.matmul(X[:, 512:1024], w1g_s[32:41, :], gta[32:41, :],
                             start=True, stop=True, tile_position=(32, 0))
            nc.tensor.matmul(X[:, 1024:1536], w1g_s[0:9, :], gtb[0:9, :],
                             start=True, stop=True, tile_position=(0, 0))
            nc.tensor.matmul(X[:, 1536:2048], w1g_s[32:41, :], gtb[32:41, :],
                             start=True, stop=True, tile_position=(32, 0))
            h1p = hp.tile([128, 2048], bf16, tag="h1", bufs=2, name="h1p")
            nc.scalar.activation(h1p[:], X[:], AF.Silu, bias=0.0)
            h1live[k] = h1p

        def l2_stage(c, h1p, hoff):
            p2 = ps.tile([128, 1024], f32, tag="p2", bufs=2, name="p2")
            nc.tensor.matmul(p2[:, 0:512], w2g_s[:],
                             h1p[:, hoff:hoff + 512],
                             start=True, stop=True, tile_position=(0, 0))
            nc.tensor.matmul(p2[:, 512:1024], w2g_s[:],
                             h1p[:, hoff + 512:hoff + 1024],
                             start=True, stop=True, tile_position=(0, 0))
            h2 = hp.tile([128, 1024], bf16, tag="h2", bufs=4, name="h2")
            nc.scalar.activation(h2[:], p2[:], AF.Silu, bias=small_s[:, 0:1])
            h2s[c] = h2

        def quad_stage(j):
            # L3 quad for tiles 4j..4j+3 into a p2-ring bank; DVE-accumulate
            h2a = h2s.pop(2 * j)
            h2b = h2s.pop(2 * j + 1)
            gi, s_ = j // 8, j % 8
            q = ps.tile([128, 1024], f32, tag="p2", bufs=2, name="q")
            for jj in range(4):
                h2 = h2a if jj < 2 else h2b
                nc.tensor.matmul(q[32 * jj:32 * jj + 32, 0:512],
                                 w3g_s[:, 32 * s_:32 * s_ + 32],
                                 h2[:, (jj % 2) * 512:(jj % 2) * 512 + 512],
                                 start=True, stop=True,
                                 tile_position=(0, 32 * jj))
            if s_ == 0:
                sgacc = hp.tile([128, KEY], f32, tag="sgacc", bufs=2,
                                name="sgacc")
                state["sgacc"] = sgacc
                nc.vector.tensor_copy(sgacc[:], q[:, 0:512])
            else:
                sgacc = state["sgacc"]
                nc.vector.tensor_add(sgacc[:], sgacc[:], q[:, 0:512])
            if s_ == 7:
                h = nc.scalar.activation(logits[:, gi * KEY:(gi + 1) * KEY],
                                         sgacc[:], AF.Silu,
                                         bias=small_s[:, 1:2])
                state["last"] = h.ins
                nc.vector.tensor_add(logits[:, gi * KEY:(gi + 1) * KEY],
                                     logits[:, gi * KEY:(gi + 1) * KEY],
                                     ty_s[:, gi * KEY:(gi + 1) * KEY])

        for c in range(3):
            dma_stage(c)
        nc.sync.dma_start(w2g_s[:], din["w2g"][:])
        nc.sync.dma_start(w3g_s[:], din["w3g"][:])
        nc.sync.dma_start(small_s[:], din["small"][:])
        for c in range(3, 6):
            dma_stage(c)
        NSS = NCH // 2
        for k in range(NSS + 2):
            if k < NSS:
                l1q_stage(k)
            if 1 <= k <= NSS:
                h1p = h1live.pop(k - 1)
                l2_stage(2 * (k - 1), h1p, 0)
                l2_stage(2 * (k - 1) + 1, h1p, 1024)
            if k == 1:
                rank_all()
            if k >= 2:
                quad_stage(k - 2)
            if k < NSS:
                for c in (2 * k + 6, 2 * k + 7):
                    if c < NCH:
                        dma_stage(c)
        last_silu = state["last"]

        # --- phase 2: exp + masked softmax-aggregate (Exp table) ---
        import os as _os
        use_dep = _os.environ.get("K_NO_DEP", "0") != "1"
        for gi in (3, 0, 1, 2):
            b = gi // 2
            e = ep.tile([128, KEY], bf16, tag="e", name="e")
            den = ep.tile([128, 1], f32, tag="den", name="den")
            h = nc.scalar.activation(e[:], logits[:, gi * KEY:(gi + 1) * KEY],
                                     AF.Exp, accum_out=den[:])
            if use_dep:
                bass_rust.add_dep_helper(h.ins, last_silu,
                                         reason="act-table phase barrier")
            eng = nc.gpsimd if gi in (0, 2) else nc.vector
            scr = ep.tile([128, KEY], bf16, tag="scr", name="scr")
            eng.tensor_mul(scr[:], e[:], fkeym_s[:, b * KEY:(b + 1) * KEY])
            num = ep.tile([128, 1], f32, tag="num", name="num")
            nc.vector.tensor_reduce(num[:], scr[:], mybir.AxisListType.X, ALU.add)
            rden = ep.tile([128, 1], f32, tag="rden", name="rden")
            nc.vector.reciprocal(rden[:], den[:])
            agg = ep.tile([128, 1], f32, tag="agg", name="agg")
            nc.vector.tensor_mul(agg[:], num[:], rden[:])
            nc.vector.scalar_tensor_tensor(
                out_s[:, gi:gi + 1], agg[:], small_s[:, 2 + gi:3 + gi],
                small_s[:, 6 + gi:7 + gi], ALU.add, ALU.mult)
        nc.sync.dma_start(dout[:], out_s[:])

    nc.compile()
    return nc


def _get_program():
    global _PROG
    if _PROG is None:
        _PROG = _build_program()
    return _PROG


def _make_in_maps(inp):
    gl, vq, cf, mask = _pack_globals(inp)
    in_maps = []
    for core in range(NCORE):
        m = dict(gl)
        m.update(_pack_core(core, inp, vq, cf, mask))
        in_maps.append({k: np.ascontiguousarray(v) for k, v in m.items()})
    return in_maps


def _unpack(res, w_out):
    cf_out = np.zeros((B, N, S, C), np.float32)
    for core in range(NCORE):
        OUT = res.results[core]["out128"]                # [128, 4]
        for gi in range(4):
            b = gi // 2
            for u in range(32):
                t = 32 * gi + u
                ql, sq = (t % 64) // 4, t % 4
                row = _row_of(u, 0)
                cf_out[b, core * QL + ql, sq, :] = OUT[row:row + 4, gi]
    return (cf_out @ w_out.T).astype(np.float32)


def kernel(**inputs) -> np.ndarray:
    from concourse.bass_utils import run_bass_kernel_spmd

    inp = {k: np.asarray(v) for k, v in inputs.items()}
    w_out = np.asarray(inp["w_out"], np.float32)
    in_maps = _make_in_maps(inp)
    nc = _get_program()
    res = run_bass_kernel_spmd(nc, in_maps, core_ids=list(range(NCORE)))
    return _unpack(res, w_out)


# revision 21
# speedup vs baseline: 1.0084x; 1.0084x over previous
"""Trainium2 Bass kernel for nn_EquivariantMultiheadAttention.

Sharding: query-point axis (dim 1) split across 8 cores (16 points each).

Structural optimizations vs the straightforward mapping:

1. ky branch as a rank-R separable expansion.  The ky-MLP is a smooth
   function of two scalars (f_key, f_query) per (batch, channel); host
   fits silu(MLP_y(fk,fq)) ~= sum_r u_r(fk) v_r(fq) via SVD on a 1-D
   grid (cubic-spline eval at data points).  On device the whole ky
   branch is ONE fp32 matmul (K = C*R+1) per 32-query-element group.
   The extra rank row carries -30*(1-mask_k), folding the key mask into
   the logits so exp() of masked keys ~ 0.

2. kg branch exact, PE-tiling aware:
   - L1 (K=9): two row-tiled matmuls per 2-tile chunk (tile_position
     (0,0)/(32,0), banded rhs) -> ~2x stream concurrency.
   - L2 (K=128 block-diag): dense matmuls, N=512 each.
   - L3 (M=32): 4-way col-tiled quads (tile_position (0,32cg),
     cg = u%4) emitted per chunk-pair -> ~4x stream concurrency.
   - Activations as [128, 1024] instructions to amortize ACT overhead.

3. Phase 2 (Exp table): exp with accum_out gives den = sum(e) free;
   num = reduce(e * fkeym) on the vector engine; residual + query mask;
   [128, 4] result.  w_out applied host-side.
"""
import numpy as np
import ml_dtypes

BF16 = ml_dtypes.bfloat16

B, N, S, DG, C, HID, COUT = 2, 128, 4, 8, 4, 32, 8
NCORE = 8
QL = N // NCORE          # 16 query points per core
KEY = N * S              # 512 keys
T = B * QL * S           # 128 tiles (query elements) per core
RK = 12                  # ky separable rank
KRANK = C * RK + 1       # 49 (last row = mask fold)
GRID = 161               # fit grid points
NCH = T // 2             # 64 two-tile chunks

_PROG = None


def _silu_np(v):
    return v / (1.0 + np.exp(-v))


def _mlp_np(x, W1, b1, W2, b2, W3, b3):
    h = _silu_np(x @ W1.T + b1)
    h = _silu_np(h @ W2.T + b2)
    return _silu_np(h @ W3.T + b3)


def _spline_eval(xg, yg, x):
    """Natural cubic spline through uniform grid (xg, yg), evaluated at x."""
    n = len(xg)
    h = float(xg[1] - xg[0])
    d = 6.0 / (h * h) * (yg[:-2] - 2.0 * yg[1:-1] + yg[2:])
    m = np.zeros(n, np.float64)
    cp = np.zeros(n - 2, np.float64)
    dp = np.zeros(n - 2, np.float64)
    cp[0] = 0.25
    dp[0] = d[0] * 0.25
    for i in range(1, n - 2):
        den = 4.0 - cp[i - 1]
        cp[i] = 1.0 / den
        dp[i] = (d[i] - dp[i - 1]) / den
    m[n - 2] = dp[-1]
    for i in range(n - 3, 0, -1):
        m[i] = dp[i - 1] - cp[i - 1] * m[i + 1]
    idx = np.clip(((x - xg[0]) / h).astype(np.int64), 0, n - 2)
    t = x - xg[idx]
    a = yg[idx]
    b_ = (yg[idx + 1] - yg[idx]) / h - h * (2.0 * m[idx] + m[idx + 1]) / 6.0
    c_ = m[idx] / 2.0
    dd = (m[idx + 1] - m[idx]) / (6.0 * h)
    return a + t * (b_ + t * (c_ + t * dd))


def _fit_ky(inp, cf):
    """Rank-RK separable factors of silu(MLP_y) per (batch, channel)."""
    ubank = np.zeros((B, C, RK, KEY), np.float32)
    vq = np.zeros((B, C, RK, N * S), np.float32)
    for b in range(B):
        for c in range(C):
            f = cf[b, :, :, c].reshape(-1).astype(np.float64)
            lo, hi = f.min(), f.max()
            pad = 0.05 * (hi - lo)
            grid = np.linspace(lo - pad, hi + pad, GRID)
            X, Y = np.meshgrid(grid, grid, indexing="ij")
            G = _mlp_np(
                np.stack([X.ravel(), Y.ravel()], -1),
                inp["ky_W1"][c], inp["ky_b1"][c], inp["ky_W2"][c],
                inp["ky_b2"][c], inp["ky_W3"][c], inp["ky_b3"][c],
            ).reshape(GRID, GRID)
            U, sv, Vt = np.linalg.svd(G)
            for r in range(RK):
                ubank[b, c, r] = _spline_eval(grid, U[:, r] * sv[r], f)
                vq[b, c, r] = _spline_eval(grid, Vt[r], f)
    return ubank, vq


def _row_of(u, c):
    """PSUM row of (tile-in-group u, channel c): 4-way col-group interleave."""
    return 32 * (u % 4) + 4 * (u // 4) + c


def _pack_globals(inp):
    cf = np.ascontiguousarray(np.asarray(inp["coset_functions"], np.float32))
    mask = np.asarray(inp["mask"]).astype(np.float32)
    out = {}

    kgW1 = np.asarray(inp["kg_W1"], np.float32)
    w1g = np.zeros((DG + 1, 128), np.float32)
    for c in range(C):
        w1g[0:DG, c * 32:(c + 1) * 32] = kgW1[c].T
    w1g[DG, :] = np.asarray(inp["kg_b1"], np.float32).reshape(128)
    w1gdup = np.zeros((128, 128), np.float32)
    for e in range(4):
        w1gdup[32 * e:32 * e + DG + 1] = w1g
    out["w1gdup"] = w1gdup.astype(BF16)

    W2 = np.asarray(inp["kg_W2"], np.float32)
    L = np.zeros((128, 128), np.float32)
    for c in range(C):
        L[c * 32:(c + 1) * 32, c * 32:(c + 1) * 32] = W2[c].T
    out["w2g"] = L.astype(BF16)

    W3g = np.asarray(inp["kg_W3"], np.float32)
    w3g = np.zeros((128, 256), np.float32)
    for s in range(8):
        for c in range(C):
            w3g[c * 32:(c + 1) * 32, 32 * s + 4 * s + c] = W3g[c, 0, :]
    out["w3g"] = w3g.astype(BF16)

    ubank, vq = _fit_ky(inp, cf)
    bkey = np.zeros((B, KRANK, KEY), np.float32)
    bkey[:, 0:C * RK, :] = ubank.reshape(B, C * RK, KEY)
    mk = mask.reshape(B, KEY)
    bkey[:, C * RK, :] = -30.0 * (1.0 - mk)
    bh = bkey.astype(BF16)
    bl = (bkey - bh.astype(np.float32)).astype(BF16)
    bkey2 = np.concatenate([bh, bl], axis=2)            # [B, KRANK, 2*KEY]
    out["bkey2"] = bkey2

    fkeym = np.zeros((B, 128, KEY), np.float32)
    for row in range(128):
        c = row % 4
        fkeym[:, row, :] = mk * cf[:, :, :, c].reshape(B, KEY)
    out["fkeym"] = fkeym.astype(BF16)
    return out, vq, cf, mask


def _pack_core(core, inp, vq, cf, mask):
    g = np.asarray(inp["pairwise_g"], np.float32)
    qs = slice(core * QL, (core + 1) * QL)
    out = {}
    # g4 [18, NCH*512]: rows 0-8 even tile (g dims + ones), rows 9-17 odd tile
    gt = g[:, qs].transpose(0, 1, 3, 5, 2, 4).reshape(T, DG, KEY)
    g4 = np.empty((18, NCH * KEY), np.float32)
    g4[0:DG] = gt[0::2].transpose(1, 0, 2).reshape(DG, NCH * KEY)
    g4[DG] = 1.0
    g4[9:9 + DG] = gt[1::2].transpose(1, 0, 2).reshape(DG, NCH * KEY)
    g4[9 + DG] = 1.0
    out["g4"] = g4.astype(BF16)

    cfq = cf[:, qs]                                      # [B,QL,S,C]
    maskq = mask[:, qs]                                  # [B,QL,S]
    b2g = np.asarray(inp["kg_b2"], np.float32).reshape(128)
    b3 = np.asarray(inp["kg_b3"], np.float32).reshape(C)

    lhsa = np.zeros((KRANK, 4 * 128), np.float32)
    lhsa[C * RK, :] = 1.0
    small = np.zeros((128, 10), np.float32)
    small[:, 0] = b2g
    for gi in range(4):
        b = gi // 2
        for u in range(32):
            t = 32 * gi + u
            ql, sq = (t % 64) // 4, t % 4
            row = _row_of(u, 0)
            qel = (core * QL + ql) * S + sq
            for c in range(C):
                lhsa[c * RK:(c + 1) * RK, gi * 128 + row + c] = vq[b, c, :, qel]
                small[row + c, 1] = b3[c]
                small[row + c, 2 + gi] = cfq[b, ql, sq, c]
                small[row + c, 6 + gi] = maskq[b, ql, sq]
    ah = lhsa.astype(BF16)
    al = (lhsa - ah.astype(np.float32)).astype(BF16)
    out["lhsa2"] = np.concatenate([ah, al], axis=1)     # [KRANK, 2*512]
    out["small"] = small
    return out


def _build_program():
    from contextlib import ExitStack
    import concourse.bass as bass
    import concourse.tile as tile
    import concourse.mybir as mybir
    from concourse import bacc
    import bass_rust

    f32 = mybir.dt.float32
    bf16 = mybir.dt.bfloat16
    AF = mybir.ActivationFunctionType
    ALU = mybir.AluOpType

    nc = bacc.Bacc("TRN2", target_bir_lowering=False, debug=False,
                   enable_asserts=False, num_devices=NCORE)

    din = {}
    for name, shape, dt in (
        ("g4", [18, NCH * KEY], bf16),
        ("w1gdup", [128, 128], bf16),
        ("w2g", [128, 128], bf16),
        ("w3g", [128, 256], bf16),
        ("bkey2", [B, KRANK, 2 * KEY], bf16),
        ("lhsa2", [KRANK, 2 * 4 * 128], bf16),
        ("small", [128, 10], f32),
        ("fkeym", [B, 128, KEY], bf16),
    ):
        din[name] = nc.dram_tensor(name, shape, dt, kind="ExternalInput").ap()
    dout = nc.dram_tensor("out128", [128, 4], f32, kind="ExternalOutput").ap()

    with tile.TileContext(nc) as tc, ExitStack() as ctx:
        const = ctx.enter_context(tc.tile_pool(name="const", bufs=1))
        gp = ctx.enter_context(tc.tile_pool(name="gp", bufs=4))
        hp = ctx.enter_context(tc.tile_pool(name="hp", bufs=2))
        ps = ctx.enter_context(tc.tile_pool(name="ps", bufs=1, space="PSUM"))
        ep = ctx.enter_context(tc.tile_pool(name="ep", bufs=2))

        # --- constants to SBUF ---
        w1g_s = const.tile([128, 128], bf16, name="w1g_s")
        nc.gpsimd.dma_start(w1g_s[:], din["w1gdup"][:])

        lhsa_s = const.tile([KRANK, 2 * 4 * 128], bf16, name="lhsa_s")
        bkey_s = const.tile([KRANK, B * 2 * KEY], bf16, name="bkey_s")
        fkeym_s = const.tile([128, B * KEY], bf16, name="fkeym_s")
        for b in range(B):
            nc.gpsimd.dma_start(bkey_s[:, b * 2 * KEY:(b + 1) * 2 * KEY],
                                din["bkey2"][b])
        nc.gpsimd.dma_start(lhsa_s[:], din["lhsa2"][:])
        for b in range(B):
            nc.gpsimd.dma_start(fkeym_s[:, b * KEY:(b + 1) * KEY], din["fkeym"][b])
        w2g_s = const.tile([128, 128], bf16, name="w2g_s")
        w3g_s = const.tile([128, 256], bf16, name="w3g_s")
        small_s = const.tile([128, 10], f32, name="small_s")
        ty_s = const.tile([128, 4 * KEY], f32, name="ty_s")
        logits = const.tile([128, 4 * KEY], f32, name="logits")
        out_s = const.tile([128, 4], f32, name="out_s")

        # --- main loop: kg MLP, software-pipelined 2-tile chunks ---
        def rank_all():
            # ky rank matmuls into p2-ring tiles (2 groups per tile)
            for half in range(2):
                Yr = ps.tile([128, 1024], f32, tag="p2", bufs=2, name="Yr")
                for q in range(2):
                    gi = 2 * half + q
                    bb = gi // 2
                    for ai, bi, st, sp in ((0, 0, True, False),
                                           (0, 1, False, False),
                                           (1, 0, False, True)):
                        nc.tensor.matmul(
                            Yr[:, q * KEY:(q + 1) * KEY],
                            lhsa_s[:, ai * 512 + gi * 128:
                                   ai * 512 + (gi + 1) * 128],
                            bkey_s[:, (2 * bb + bi) * KEY:
                                   (2 * bb + bi + 1) * KEY],
                            start=st, stop=sp, tile_position=(0, 0))
                for q in range(2):
                    gi = 2 * half + q
                    nc.vector.tensor_copy(ty_s[:, gi * KEY:(gi + 1) * KEY],
                                          Yr[:, q * KEY:(q + 1) * KEY])

        gts = {}

        def dma_stage(c):
            gt = gp.tile([41, KEY], bf16, tag="gt", bufs=6, name="gt")
            nc.sync.dma_start(gt[0:9, :], din["g4"][0:9, c * KEY:(c + 1) * KEY])
            nc.sync.dma_start(gt[32:41, :],
                              din["g4"][9:18, c * KEY:(c + 1) * KEY])
            gts[c] = gt

        h1live = {}
        h2s = {}
        Xs = {}
        state = {"last": None, "sgacc": None}

        def l1q_stage(k):
            # L1 for chunks 2k, 2k+1 (tiles 4k..4k+3) + fused [128,2048] ACT
            gta = gts.pop(2 * k)
            gtb = gts.pop(2 * k + 1)
            X = ps.tile([128, 2048], f32, tag="px", bufs=1, name="X")
            nc.tensor.matmul(X[:, 0:512], w1g_s[0:9, :], gta[0:9, :],
                             start=True, stop=True, tile_position=(0, 0))
            nc.tensor.matmul(X[:, 512:1024], w1g_s[32:41, :], gta[32:41, :],
                             start=True, stop=True, tile_position=(32, 0))
            nc.tensor.matmul(X[:, 1024:1536], w1g_s[0:9, :], gtb[0:9, :],
                             start=True, stop=True, tile_position=(0, 0))
            nc.tensor.matmul(X[:, 1536:2048], w1g_s[32:41, :], gtb[32:41, :],
                             start=True, stop=True, tile_position=(32, 0))
            h1p = hp.tile([128, 2048], bf16, tag="h1", bufs=2, name="h1p")
            nc.scalar.activation(h1p[:], X[:], AF.Silu, bias=0.0)
            h1live[k] = h1p

        def l2_stage(c, h1p, hoff):
            p2 = ps.tile([128, 1024], f32, tag="p2", bufs=2, name="p2")
            nc.tensor.matmul(p2[:, 0:512], w2g_s[:],
                             h1p[:, hoff:hoff + 512],
                             start=True, stop=True, tile_position=(0, 0))
            nc.tensor.matmul(p2[:, 512:1024], w2g_s[:],
                             h1p[:, hoff + 512:hoff + 1024],
                             start=True, stop=True, tile_position=(0, 0))
            h2 = hp.tile([128, 1024], bf16, tag="h2", bufs=4, name="h2")
            nc.scalar.activation(h2[:], p2[:], AF.Silu, bias=small_s[:, 0:1])
            h2s[c] = h2

        def quad_stage(j):
            # L3 quad for tiles 4j..4j+3 into a p2-ring bank; DVE-accumulate
            h2a = h2s.pop(2 * j)
            h2b = h2s.pop(2 * j + 1)
            gi, s_ = j // 8, j % 8
            q = ps.tile([128, 1024], f32, tag="p2", bufs=2, name="q")
            for jj in range(4):
                h2 = h2a if jj < 2 else h2b
                nc.tensor.matmul(q[32 * jj:32 * jj + 32, 0:512],
                                 w3g_s[:, 32 * s_:32 * s_ + 32],
                                 h2[:, (jj % 2) * 512:(jj % 2) * 512 + 512],
                                 start=True, stop=True,
                                 tile_position=(0, 32 * jj))
            if s_ == 0:
                sgacc = hp.tile([128, KEY], f32, tag="sgacc", bufs=2,
                                name="sgacc")
                state["sgacc"] = sgacc
                nc.vector.tensor_copy(sgacc[:], q[:, 0:512])
            else:
                sgacc = state["sgacc"]
                nc.vector.tensor_add(sgacc[:], sgacc[:], q[:, 0:512])
            if s_ == 7:
                h = nc.scalar.activation(logits[:, gi * KEY:(gi + 1) * KEY],
                                         sgacc[:], AF.Silu,
                                         bias=small_s[:, 1:2])
                state["last"] = h.ins
                nc.vector.tensor_add(logits[:, gi * KEY:(gi + 1) * KEY],
                                     logits[:, gi * KEY:(gi + 1) * KEY],
                                     ty_s[:, gi * KEY:(gi + 1) * KEY])

        for c in range(3):
            dma_stage(c)
        nc.scalar.dma_start(w2g_s[:], din["w2g"][:])
        nc.scalar.dma_start(w3g_s[:], din["w3g"][:])
        nc.scalar.dma_start(small_s[:], din["small"][:])
        for c in range(3, 6):
            dma_stage(c)
        NSS = NCH // 2
        for k in range(NSS + 2):
            if k < NSS:
                l1q_stage(k)
            if 1 <= k <= NSS:
                h1p = h1live.pop(k - 1)
                l2_stage(2 * (k - 1), h1p, 0)
                l2_stage(2 * (k - 1) + 1, h1p, 1024)
            if k == 1:
                rank_all()
            if k >= 2:
                quad_stage(k - 2)
            if k < NSS:
                for c in (2 * k + 6, 2 * k + 7):
                    if c < NCH:
                        dma_stage(c)
        last_silu = state["last"]

        # --- phase 2: exp + masked softmax-aggregate (Exp table) ---
        import os as _os
        use_dep = _os.environ.get("K_NO_DEP", "0") != "1"
        for gi in (3, 0, 1, 2):
            b = gi // 2
            e = ep.tile([128, KEY], bf16, tag="e", name="e")
            den = ep.tile([128, 1], f32, tag="den", name="den")
            h = nc.scalar.activation(e[:], logits[:, gi * KEY:(gi + 1) * KEY],
                                     AF.Exp, accum_out=den[:])
            if use_dep:
                bass_rust.add_dep_helper(h.ins, last_silu,
                                         reason="act-table phase barrier")
            eng = nc.gpsimd if gi in (0, 2) else nc.vector
            scr = ep.tile([128, KEY], bf16, tag="scr", name="scr")
            eng.tensor_mul(scr[:], e[:], fkeym_s[:, b * KEY:(b + 1) * KEY])
            num = ep.tile([128, 1], f32, tag="num", name="num")
            nc.vector.tensor_reduce(num[:], scr[:], mybir.AxisListType.X, ALU.add)
            rden = ep.tile([128, 1], f32, tag="rden", name="rden")
            nc.vector.reciprocal(rden[:], den[:])
            agg = ep.tile([128, 1], f32, tag="agg", name="agg")
            nc.vector.tensor_mul(agg[:], num[:], rden[:])
            nc.vector.scalar_tensor_tensor(
                out_s[:, gi:gi + 1], agg[:], small_s[:, 2 + gi:3 + gi],
                small_s[:, 6 + gi:7 + gi], ALU.add, ALU.mult)
        nc.sync.dma_start(dout[:], out_s[:])

    nc.compile()
    return nc


def _get_program():
    global _PROG
    if _PROG is None:
        _PROG = _build_program()
    return _PROG


def _make_in_maps(inp):
    gl, vq, cf, mask = _pack_globals(inp)
    in_maps = []
    for core in range(NCORE):
        m = dict(gl)
        m.update(_pack_core(core, inp, vq, cf, mask))
        in_maps.append({k: np.ascontiguousarray(v) for k, v in m.items()})
    return in_maps


def _unpack(res, w_out):
    cf_out = np.zeros((B, N, S, C), np.float32)
    for core in range(NCORE):
        OUT = res.results[core]["out128"]                # [128, 4]
        for gi in range(4):
            b = gi // 2
            for u in range(32):
                t = 32 * gi + u
                ql, sq = (t % 64) // 4, t % 4
                row = _row_of(u, 0)
                cf_out[b, core * QL + ql, sq, :] = OUT[row:row + 4, gi]
    return (cf_out @ w_out.T).astype(np.float32)


def kernel(**inputs) -> np.ndarray:
    from concourse.bass_utils import run_bass_kernel_spmd

    inp = {k: np.asarray(v) for k, v in inputs.items()}
    w_out = np.asarray(inp["w_out"], np.float32)
    in_maps = _make_in_maps(inp)
    nc = _get_program()
    res = run_bass_kernel_spmd(nc, in_maps, core_ids=list(range(NCORE)))
    return _unpack(res, w_out)


# revision 23
# speedup vs baseline: 1.0268x; 1.0183x over previous
"""Trainium2 Bass kernel for nn_EquivariantMultiheadAttention.

Sharding: query-point axis (dim 1) split across 8 cores (16 points each).

Structural optimizations vs the straightforward mapping:

1. ky branch as a rank-R separable expansion.  The ky-MLP is a smooth
   function of two scalars (f_key, f_query) per (batch, channel); host
   fits silu(MLP_y(fk,fq)) ~= sum_r u_r(fk) v_r(fq) via SVD on a 1-D
   grid (cubic-spline eval at data points).  On device the whole ky
   branch is ONE fp32 matmul (K = C*R+1) per 32-query-element group.
   The extra rank row carries -30*(1-mask_k), folding the key mask into
   the logits so exp() of masked keys ~ 0.

2. kg branch exact, PE-tiling aware:
   - L1 (K=9): two row-tiled matmuls per 2-tile chunk (tile_position
     (0,0)/(32,0), banded rhs) -> ~2x stream concurrency.
   - L2 (K=128 block-diag): dense matmuls, N=512 each.
   - L3 (M=32): 4-way col-tiled quads (tile_position (0,32cg),
     cg = u%4) emitted per chunk-pair -> ~4x stream concurrency.
   - Activations as [128, 1024] instructions to amortize ACT overhead.

3. Phase 2 (Exp table): exp with accum_out gives den = sum(e) free;
   num = reduce(e * fkeym) on the vector engine; residual + query mask;
   [128, 4] result.  w_out applied host-side.
"""
import numpy as np
import ml_dtypes

BF16 = ml_dtypes.bfloat16

B, N, S, DG, C, HID, COUT = 2, 128, 4, 8, 4, 32, 8
NCORE = 8
QL = N // NCORE          # 16 query points per core
KEY = N * S              # 512 keys
T = B * QL * S           # 128 tiles (query elements) per core
RK = 12                  # ky separable rank
KRANK = C * RK + 1       # 49 (last row = mask fold)
GRID = 161               # fit grid points
NCH = T // 2             # 64 two-tile chunks

_PROG = None


def _silu_np(v):
    return v / (1.0 + np.exp(-v))


def _mlp_np(x, W1, b1, W2, b2, W3, b3):
    h = _silu_np(x @ W1.T + b1)
    h = _silu_np(h @ W2.T + b2)
    return _silu_np(h @ W3.T + b3)


def _spline_eval(xg, yg, x):
    """Natural cubic spline through uniform grid (xg, yg), evaluated at x."""
    n = len(xg)
    h = float(xg[1] - xg[0])
    d = 6.0 / (h * h) * (yg[:-2] - 2.0 * yg[1:-1] + yg[2:])
    m = np.zeros(n, np.float64)
    cp = np.zeros(n - 2, np.float64)
    dp = np.zeros(n - 2, np.float64)
    cp[0] = 0.25
    dp[0] = d[0] * 0.25
    for i in range(1, n - 2):
        den = 4.0 - cp[i - 1]
        cp[i] = 1.0 / den
        dp[i] = (d[i] - dp[i - 1]) / den
    m[n - 2] = dp[-1]
    for i in range(n - 3, 0, -1):
        m[i] = dp[i - 1] - cp[i - 1] * m[i + 1]
    idx = np.clip(((x - xg[0]) / h).astype(np.int64), 0, n - 2)
    t = x - xg[idx]
    a = yg[idx]
    b_ = (yg[idx + 1] - yg[idx]) / h - h * (2.0 * m[idx] + m[idx + 1]) / 6.0
    c_ = m[idx] / 2.0
    dd = (m[idx + 1] - m[idx]) / (6.0 * h)
    return a + t * (b_ + t * (c_ + t * dd))


def _fit_ky(inp, cf):
    """Rank-RK separable factors of silu(MLP_y) per (batch, channel)."""
    ubank = np.zeros((B, C, RK, KEY), np.float32)
    vq = np.zeros((B, C, RK, N * S), np.float32)
    for b in range(B):
        for c in range(C):
            f = cf[b, :, :, c].reshape(-1).astype(np.float64)
            lo, hi = f.min(), f.max()
            pad = 0.05 * (hi - lo)
            grid = np.linspace(lo - pad, hi + pad, GRID)
            X, Y = np.meshgrid(grid, grid, indexing="ij")
            G = _mlp_np(
                np.stack([X.ravel(), Y.ravel()], -1),
                inp["ky_W1"][c], inp["ky_b1"][c], inp["ky_W2"][c],
                inp["ky_b2"][c], inp["ky_W3"][c], inp["ky_b3"][c],
            ).reshape(GRID, GRID)
            U, sv, Vt = np.linalg.svd(G)
            for r in range(RK):
                ubank[b, c, r] = _spline_eval(grid, U[:, r] * sv[r], f)
                vq[b, c, r] = _spline_eval(grid, Vt[r], f)
    return ubank, vq


def _row_of(u, c):
    """PSUM row of (tile-in-group u, channel c): 4-way col-group interleave."""
    return 32 * (u % 4) + 4 * (u // 4) + c


def _pack_globals(inp):
    cf = np.ascontiguousarray(np.asarray(inp["coset_functions"], np.float32))
    mask = np.asarray(inp["mask"]).astype(np.float32)
    out = {}

    kgW1 = np.asarray(inp["kg_W1"], np.float32)
    w1g = np.zeros((DG + 1, 128), np.float32)
    for c in range(C):
        w1g[0:DG, c * 32:(c + 1) * 32] = kgW1[c].T
    w1g[DG, :] = np.asarray(inp["kg_b1"], np.float32).reshape(128)
    w1gdup = np.zeros((128, 128), np.float32)
    for e in range(4):
        w1gdup[32 * e:32 * e + DG + 1] = w1g
    out["w1gdup"] = w1gdup.astype(BF16)

    W2 = np.asarray(inp["kg_W2"], np.float32)
    L = np.zeros((128, 128), np.float32)
    for c in range(C):
        L[c * 32:(c + 1) * 32, c * 32:(c + 1) * 32] = W2[c].T
    out["w2g"] = L.astype(BF16)

    W3g = np.asarray(inp["kg_W3"], np.float32)
    w3g = np.zeros((128, 256), np.float32)
    for s in range(8):
        for c in range(C):
            w3g[c * 32:(c + 1) * 32, 32 * s + 4 * s + c] = W3g[c, 0, :]
    out["w3g"] = w3g.astype(BF16)

    ubank, vq = _fit_ky(inp, cf)
    bkey = np.zeros((B, KRANK, KEY), np.float32)
    bkey[:, 0:C * RK, :] = ubank.reshape(B, C * RK, KEY)
    mk = mask.reshape(B, KEY)
    bkey[:, C * RK, :] = -30.0 * (1.0 - mk)
    bh = bkey.astype(BF16)
    bl = (bkey - bh.astype(np.float32)).astype(BF16)
    bkey2 = np.concatenate([bh, bl], axis=2)            # [B, KRANK, 2*KEY]
    out["bkey2"] = bkey2

    fkeym = np.zeros((B, 128, KEY), np.float32)
    for row in range(128):
        c = row % 4
        fkeym[:, row, :] = mk * cf[:, :, :, c].reshape(B, KEY)
    out["fkeym"] = fkeym.astype(BF16)
    return out, vq, cf, mask


def _pack_core(core, inp, vq, cf, mask):
    g = np.asarray(inp["pairwise_g"], np.float32)
    qs = slice(core * QL, (core + 1) * QL)
    out = {}
    # g4 [18, NCH*512]: rows 0-8 even tile (g dims + ones), rows 9-17 odd tile
    gt = g[:, qs].transpose(0, 1, 3, 5, 2, 4).reshape(T, DG, KEY)
    g4 = np.empty((18, NCH * KEY), np.float32)
    g4[0:DG] = gt[0::2].transpose(1, 0, 2).reshape(DG, NCH * KEY)
    g4[DG] = 1.0
    g4[9:9 + DG] = gt[1::2].transpose(1, 0, 2).reshape(DG, NCH * KEY)
    g4[9 + DG] = 1.0
    out["g4"] = g4.astype(BF16)

    cfq = cf[:, qs]                                      # [B,QL,S,C]
    maskq = mask[:, qs]                                  # [B,QL,S]
    b2g = np.asarray(inp["kg_b2"], np.float32).reshape(128)
    b3 = np.asarray(inp["kg_b3"], np.float32).reshape(C)

    lhsa = np.zeros((KRANK, 4 * 128), np.float32)
    lhsa[C * RK, :] = 1.0
    small = np.zeros((128, 10), np.float32)
    small[:, 0] = b2g
    for gi in range(4):
        b = gi // 2
        for u in range(32):
            t = 32 * gi + u
            ql, sq = (t % 64) // 4, t % 4
            row = _row_of(u, 0)
            qel = (core * QL + ql) * S + sq
            for c in range(C):
                lhsa[c * RK:(c + 1) * RK, gi * 128 + row + c] = vq[b, c, :, qel]
                small[row + c, 1] = b3[c]
                small[row + c, 2 + gi] = cfq[b, ql, sq, c]
                small[row + c, 6 + gi] = maskq[b, ql, sq]
    ah = lhsa.astype(BF16)
    al = (lhsa - ah.astype(np.float32)).astype(BF16)
    out["lhsa2"] = np.concatenate([ah, al], axis=1)     # [KRANK, 2*512]
    out["small"] = small
    return out


def _build_program():
    from contextlib import ExitStack
    import concourse.bass as bass
    import concourse.tile as tile
    import concourse.mybir as mybir
    from concourse import bacc
    import bass_rust

    f32 = mybir.dt.float32
    bf16 = mybir.dt.bfloat16
    AF = mybir.ActivationFunctionType
    ALU = mybir.AluOpType

    nc = bacc.Bacc("TRN2", target_bir_lowering=False, debug=False,
                   enable_asserts=False, num_devices=NCORE)

    din = {}
    for name, shape, dt in (
        ("g4", [18, NCH * KEY], bf16),
        ("w1gdup", [128, 128], bf16),
        ("w2g", [128, 128], bf16),
        ("w3g", [128, 256], bf16),
        ("bkey2", [B, KRANK, 2 * KEY], bf16),
        ("lhsa2", [KRANK, 2 * 4 * 128], bf16),
        ("small", [128, 10], f32),
        ("fkeym", [B, 128, KEY], bf16),
    ):
        din[name] = nc.dram_tensor(name, shape, dt, kind="ExternalInput").ap()
    dout = nc.dram_tensor("out128", [128, 4], f32, kind="ExternalOutput").ap()

    with tile.TileContext(nc) as tc, ExitStack() as ctx:
        const = ctx.enter_context(tc.tile_pool(name="const", bufs=1))
        gp = ctx.enter_context(tc.tile_pool(name="gp", bufs=4))
        hp = ctx.enter_context(tc.tile_pool(name="hp", bufs=2))
        ps = ctx.enter_context(tc.tile_pool(name="ps", bufs=1, space="PSUM"))
        ep = ctx.enter_context(tc.tile_pool(name="ep", bufs=2))

        # --- constants to SBUF ---
        w1g_s = const.tile([128, 128], bf16, name="w1g_s")
        nc.gpsimd.dma_start(w1g_s[:], din["w1gdup"][:])

        lhsa_s = const.tile([KRANK, 2 * 4 * 128], bf16, name="lhsa_s")
        bkey_s = const.tile([KRANK, B * 2 * KEY], bf16, name="bkey_s")
        fkeym_s = const.tile([128, B * KEY], bf16, name="fkeym_s")
        for b in range(B):
            nc.gpsimd.dma_start(bkey_s[:, b * 2 * KEY:(b + 1) * 2 * KEY],
                                din["bkey2"][b])
        nc.gpsimd.dma_start(lhsa_s[:], din["lhsa2"][:])
        for b in range(B):
            nc.gpsimd.dma_start(fkeym_s[:, b * KEY:(b + 1) * KEY], din["fkeym"][b])
        w2g_s = const.tile([128, 128], bf16, name="w2g_s")
        w3g_s = const.tile([128, 256], bf16, name="w3g_s")
        small_s = const.tile([128, 10], f32, name="small_s")
        ty_s = const.tile([128, 4 * KEY], f32, name="ty_s")
        logits = const.tile([128, 4 * KEY], f32, name="logits")
        out_s = const.tile([128, 4], f32, name="out_s")

        # --- main loop: kg MLP, software-pipelined 2-tile chunks ---
        rank_yr = {}

        def rank_stage(step):
            # one bf16 rank matmul per step; triple accumulates per group
            gi, part = step // 3, step % 3
            ai, bi, st, sp = ((0, 0, True, False), (0, 1, False, False),
                              (1, 0, False, True))[part]
            if part == 0:
                rank_yr[gi] = ps.tile([128, 1024], f32, tag="p2", bufs=2,
                                      name="Yr")
            Yr = rank_yr[gi]
            bb = gi // 2
            nc.tensor.matmul(
                Yr[:, 0:KEY],
                lhsa_s[:, ai * 512 + gi * 128:ai * 512 + (gi + 1) * 128],
                bkey_s[:, (2 * bb + bi) * KEY:(2 * bb + bi + 1) * KEY],
                start=st, stop=sp, tile_position=(0, 0))
            if part == 2:
                nc.vector.tensor_copy(ty_s[:, gi * KEY:(gi + 1) * KEY],
                                      Yr[:, 0:KEY])
                rank_yr.pop(gi)

        gts = {}

        def dma_stage(c):
            gt = gp.tile([41, KEY], bf16, tag="gt", bufs=6, name="gt")
            nc.sync.dma_start(gt[0:9, :], din["g4"][0:9, c * KEY:(c + 1) * KEY])
            nc.sync.dma_start(gt[32:41, :],
                              din["g4"][9:18, c * KEY:(c + 1) * KEY])
            gts[c] = gt

        h1live = {}
        h2s = {}
        Xs = {}
        state = {"last": None, "sgacc": None}

        def l1q_stage(k):
            # L1 for chunks 2k, 2k+1 (tiles 4k..4k+3) + fused [128,2048] ACT
            gta = gts.pop(2 * k)
            gtb = gts.pop(2 * k + 1)
            X = ps.tile([128, 2048], f32, tag="px", bufs=1, name="X")
            nc.tensor.matmul(X[:, 0:512], w1g_s[0:9, :], gta[0:9, :],
                             start=True, stop=True, tile_position=(0, 0))
            nc.tensor.matmul(X[:, 512:1024], w1g_s[32:41, :], gta[32:41, :],
                             start=True, stop=True, tile_position=(32, 0))
            nc.tensor.matmul(X[:, 1024:1536], w1g_s[0:9, :], gtb[0:9, :],
                             start=True, stop=True, tile_position=(0, 0))
            nc.tensor.matmul(X[:, 1536:2048], w1g_s[32:41, :], gtb[32:41, :],
                             start=True, stop=True, tile_position=(32, 0))
            h1p = hp.tile([128, 2048], bf16, tag="h1", bufs=2, name="h1p")
            nc.scalar.activation(h1p[:], X[:], AF.Silu, bias=0.0)
            h1live[k] = h1p

        def l2_stage(c, h1p, hoff):
            p2 = ps.tile([128, 1024], f32, tag="p2", bufs=2, name="p2")
            nc.tensor.matmul(p2[:, 0:512], w2g_s[:],
                             h1p[:, hoff:hoff + 512],
                             start=True, stop=True, tile_position=(0, 0))
            nc.tensor.matmul(p2[:, 512:1024], w2g_s[:],
                             h1p[:, hoff + 512:hoff + 1024],
                             start=True, stop=True, tile_position=(0, 0))
            h2 = hp.tile([128, 1024], bf16, tag="h2", bufs=4, name="h2")
            nc.scalar.activation(h2[:], p2[:], AF.Silu, bias=small_s[:, 0:1])
            h2s[c] = h2

        def quad_stage(j):
            # L3 quad for tiles 4j..4j+3 into a p2-ring bank; DVE-accumulate
            h2a = h2s.pop(2 * j)
            h2b = h2s.pop(2 * j + 1)
            gi, s_ = j // 8, j % 8
            q = ps.tile([128, 1024], f32, tag="p2", bufs=2, name="q")
            for jj in range(4):
                h2 = h2a if jj < 2 else h2b
                nc.tensor.matmul(q[32 * jj:32 * jj + 32, 0:512],
                                 w3g_s[:, 32 * s_:32 * s_ + 32],
                                 h2[:, (jj % 2) * 512:(jj % 2) * 512 + 512],
                                 start=True, stop=True,
                                 tile_position=(0, 32 * jj))
            if s_ == 0:
                sgacc = hp.tile([128, KEY], f32, tag="sgacc", bufs=2,
                                name="sgacc")
                state["sgacc"] = sgacc
                nc.vector.tensor_copy(sgacc[:], q[:, 0:512])
            else:
                sgacc = state["sgacc"]
                nc.vector.tensor_add(sgacc[:], sgacc[:], q[:, 0:512])
            if s_ == 7:
                h = nc.scalar.activation(logits[:, gi * KEY:(gi + 1) * KEY],
                                         sgacc[:], AF.Silu,
                                         bias=small_s[:, 1:2])
                state["last"] = h.ins
                nc.vector.tensor_add(logits[:, gi * KEY:(gi + 1) * KEY],
                                     logits[:, gi * KEY:(gi + 1) * KEY],
                                     ty_s[:, gi * KEY:(gi + 1) * KEY])

        for c in range(3):
            dma_stage(c)
        nc.scalar.dma_start(w2g_s[:], din["w2g"][:])
        nc.scalar.dma_start(w3g_s[:], din["w3g"][:])
        nc.scalar.dma_start(small_s[:], din["small"][:])
        for c in range(3, 6):
            dma_stage(c)
        NSS = NCH // 2
        for k in range(NSS + 2):
            if k < NSS:
                l1q_stage(k)
            if 1 <= k <= NSS:
                h1p = h1live.pop(k - 1)
                l2_stage(2 * (k - 1), h1p, 0)
                l2_stage(2 * (k - 1) + 1, h1p, 1024)
            if 1 <= k <= 12:
                rank_stage(k - 1)
            if k >= 2:
                quad_stage(k - 2)
            if k < NSS:
                for c in (2 * k + 6, 2 * k + 7):
                    if c < NCH:
                        dma_stage(c)
        last_silu = state["last"]

        # --- phase 2: exp + masked softmax-aggregate (Exp table) ---
        import os as _os
        use_dep = _os.environ.get("K_NO_DEP", "0") != "1"
        for gi in (3, 0, 1, 2):
            b = gi // 2
            e = ep.tile([128, KEY], bf16, tag="e", name="e")
            den = ep.tile([128, 1], f32, tag="den", name="den")
            h = nc.scalar.activation(e[:], logits[:, gi * KEY:(gi + 1) * KEY],
                                     AF.Exp, accum_out=den[:])
            if use_dep:
                bass_rust.add_dep_helper(h.ins, last_silu,
                                         reason="act-table phase barrier")
            eng = nc.gpsimd if gi in (0, 2) else nc.vector
            scr = ep.tile([128, KEY], bf16, tag="scr", name="scr")
            eng.tensor_mul(scr[:], e[:], fkeym_s[:, b * KEY:(b + 1) * KEY])
            num = ep.tile([128, 1], f32, tag="num", name="num")
            nc.vector.tensor_reduce(num[:], scr[:], mybir.AxisListType.X, ALU.add)
            rden = ep.tile([128, 1], f32, tag="rden", name="rden")
            nc.vector.reciprocal(rden[:], den[:])
            agg = ep.tile([128, 1], f32, tag="agg", name="agg")
            nc.vector.tensor_mul(agg[:], num[:], rden[:])
            nc.vector.scalar_tensor_tensor(
                out_s[:, gi:gi + 1], agg[:], small_s[:, 2 + gi:3 + gi],
                small_s[:, 6 + gi:7 + gi], ALU.add, ALU.mult)
        nc.sync.dma_start(dout[:], out_s[:])

    nc.compile()
    return nc


def _get_program():
    global _PROG
    if _PROG is None:
        _PROG = _build_program()
    return _PROG


def _make_in_maps(inp):
    gl, vq, cf, mask = _pack_globals(inp)
    in_maps = []
    for core in range(NCORE):
        m = dict(gl)
        m.update(_pack_core(core, inp, vq, cf, mask))
        in_maps.append({k: np.ascontiguousarray(v) for k, v in m.items()})
    return in_maps


def _unpack(res, w_out):
    cf_out = np.zeros((B, N, S, C), np.float32)
    for core in range(NCORE):
        OUT = res.results[core]["out128"]                # [128, 4]
        for gi in range(4):
            b = gi // 2
            for u in range(32):
                t = 32 * gi + u
                ql, sq = (t % 64) // 4, t % 4
                row = _row_of(u, 0)
                cf_out[b, core * QL + ql, sq, :] = OUT[row:row + 4, gi]
    return (cf_out @ w_out.T).astype(np.float32)


def kernel(**inputs) -> np.ndarray:
    from concourse.bass_utils import run_bass_kernel_spmd

    inp = {k: np.asarray(v) for k, v in inputs.items()}
    w_out = np.asarray(inp["w_out"], np.float32)
    in_maps = _make_in_maps(inp)
    nc = _get_program()
    res = run_bass_kernel_spmd(nc, in_maps, core_ids=list(range(NCORE)))
    return _unpack(res, w_out)
